# revision 53
# baseline (speedup 1.0000x reference)
"""Trainium2 Bass kernel for nn_Block_8916352107084 (dense transformer block).

Sharding: 8 cores = 4 batches x 2 zigzag row-quarters. Core h of a batch owns
global rows [256h, 256h+256) u [512+256h, 512+256h+256), so both cores of a
pair carry the same causal workload. Keys are permuted per core to
[other_lo | other_hi | own_lo | own_hi]; the own window is columns 512:1024.

Precision plan (error ledger, rel to 2e-2 gate):
  - x streamed in bf16; x8 = fp8(32x) feeds Q/K/V fp8 DoubleRow matmuls
    (256-deep contraction per pass => half the passes of f32r).
  - scores in f32r with 64-partition operands (qt/kt sliced per head via
    base-partition/tile_position; no zero-padded qz).
  - probs: 3 grouped exps per head into [128, 4, 256] fp8 slabs
    (gated / plain / diag); the diagonal causal mask is added into the
    scores PSUM as -1e30 via an identity x step-pattern matmul, so no
    post-exp mask multiply; softmax denominators via ones-augmented V.
  - AV in fp8 DoubleRow over adjacent key-stile slab pairs.
  - proj in f32r; FFN entirely in bf16 (w2 contraction in one PSUM group).

Self-contained: hardcodes shapes from the problem spec.
"""

import numpy as np
from contextlib import ExitStack

import jax
import concourse.bass as bass
import concourse.tile as tile
from concourse import bacc, mybir
from concourse import bass2jax

F32 = mybir.dt.float32
F32R = mybir.dt.float32r
F8 = mybir.dt.float8e4
BF16 = mybir.dt.bfloat16
DR = mybir.MatmulPerfMode.DoubleRow
A = mybir.ActivationFunctionType
ALU = mybir.AluOpType

B, T, C, H = 4, 1024, 1024, 16
DH = C // H          # 64
DFF = 4 * C          # 4096
EPS = 1e-6
SCALE = float(C) ** -0.5  # 1/32

N_CORES = 8
R = 512
CT = C // 128
ST = T // 128
MT = CT
FT = DFF // 128
TCH = T // 512

TD = 10
TE = 10
TP = 10               # fp8 probs-slab pool [128, 4, 256]
CP = CT // 2          # 4 c-chunk pairs for DoubleRow
SH2 = 32.0            # x prescale folded into rsqrt (fp8 range hygiene)


def build_module(repeat: int = 1, phase: str = "full"):
    nc = bacc.Bacc("TRN2", target_bir_lowering=False, debug=False,
                   num_devices=N_CORES)
    ap = lambda t: t.ap()
    xT = ap(nc.dram_tensor("xT", [C, T], BF16, kind="ExternalInput"))
    wq8 = ap(nc.dram_tensor("wq8", [MT, 128, CP, 2, 128], F8,
                            kind="ExternalInput"))
    wk8 = ap(nc.dram_tensor("wk8", [MT, 128, CP, 2, 128], F8,
                            kind="ExternalInput"))
    wv8 = ap(nc.dram_tensor("wv8", [TCH, 128, CP, 2, 512], F8,
                            kind="ExternalInput"))
    qsc = ap(nc.dram_tensor("qsc", [1], F32, kind="ExternalInput"))
    wpb = ap(nc.dram_tensor("wpb", [MT, 128, CT, 128], BF16,
                            kind="ExternalInput"))
    w1b = ap(nc.dram_tensor("w1b", [FT, 128, CT, 128], BF16,
                            kind="ExternalInput"))
    w2b = ap(nc.dram_tensor("w2b", [MT, 128, FT, 128], BF16,
                            kind="ExternalInput"))
    bp = ap(nc.dram_tensor("bp", [C], F32, kind="ExternalInput"))
    b1 = ap(nc.dram_tensor("b1", [DFF], F32, kind="ExternalInput"))
    b2 = ap(nc.dram_tensor("b2", [C], F32, kind="ExternalInput"))
    ident = ap(nc.dram_tensor("ident", [128, 128], F32, kind="ExternalInput"))
    stepm = ap(nc.dram_tensor("stepm", [128, 512], F32, kind="ExternalInput"))
    mbias = ap(nc.dram_tensor("mbias", [128, ST], F32, kind="ExternalInput"))
    ones = ap(nc.dram_tensor("ones", [128], F32, kind="ExternalInput"))
    outT = ap(nc.dram_tensor("outT", [C, R], F32, kind="ExternalOutput"))

    with tile.TileContext(nc) as tc, ExitStack() as ctx:
        ctx.enter_context(nc.allow_low_precision(reason="fp8/bf16 matmuls"))
        sb = ctx.enter_context(tc.tile_pool(name="sb", bufs=1))
        ps = ctx.enter_context(tc.tile_pool(name="ps", bufs=1, space="PSUM"))

        def tD(name):
            return sb.tile([128, R], F32R, tag="tD", bufs=TD, name=name)

        def tE(name):
            return sb.tile([128, R], F32, tag="tE", bufs=TE, name=name)

        def tP(name):
            return sb.tile([128, 4, 256], F8, tag="tP", bufs=TP, name=name)

        def scb(name):
            return ps.tile([128, 1024], F32, tag="scb", bufs=2, name=name)

        def body():
            ones_col = sb.tile([128, 1], F32R, tag="ones_col", bufs=1)
            nc.sync.dma_start(ones_col[:],
                              ones.rearrange("(p o) -> p o", o=1).bitcast(F32R))
            ones_row = sb.tile([1, 128], F32R, tag="ones_row", bufs=1)
            nc.sync.dma_start(ones_row[:],
                              ones.rearrange("(o f) -> o f", o=1).bitcast(F32R))
            bp_t = sb.tile([128, MT], F32, tag="bp", bufs=1)
            nc.sync.dma_start(bp_t[:], bp.rearrange("(m p) -> p m", p=128))
            b1_t = sb.tile([128, FT], F32, tag="b1", bufs=1)
            nc.sync.dma_start(b1_t[:], b1.rearrange("(m p) -> p m", p=128))
            b2_t = sb.tile([128, MT], F32, tag="b2", bufs=1)
            nc.sync.dma_start(b2_t[:], b2.rearrange("(m p) -> p m", p=128))
            id_t = sb.tile([128, 128], F32R, tag="ident", bufs=1)
            nc.sync.dma_start(id_t[:], ident.bitcast(F32R))
            step_t = sb.tile([128, 512], F32R, tag="stepm", bufs=1)
            nc.sync.dma_start(step_t[:], stepm.bitcast(F32R))
            mbias_t = sb.tile([128, ST], F32, tag="mbias", bufs=1)
            nc.sync.dma_start(mbias_t[:], mbias[:])
            qsc_t = sb.tile([1, 1], F32, tag="qsc", bufs=1)
            nc.sync.dma_start(qsc_t[:], qsc.rearrange("(o f) -> o f", o=1))

            xt = []
            for c in range(CT):
                t = sb.tile([128, T], BF16, tag="xt", bufs=CT, name=f"xt{c}")
                nc.sync.dma_start(t[:], xT[128 * c:128 * (c + 1), :])
                xt.append(t)
            xtr = [xt[c][:, 512:1024] for c in range(CT)]
            # x8 = fp8(SH2 * x), c-chunk pairs along DR planes
            x8 = [sb.tile([128, 2, T], F8, tag="x8", bufs=CP, name=f"x8_{j}")
                  for j in range(CP)]
            for c in range(CT):
                nc.scalar.activation(x8[c // 2][:, c % 2, :], xt[c][:],
                                     A.Copy, scale=SH2)
            x8r = [x8[j][:, :, 512:1024] for j in range(CP)]

            ssq = [scb(f"ssq{i}") for i in range(TCH)]
            for c in range(CT):
                for ch in range(TCH):
                    sq = tD(f"sq{c}_{ch}")
                    nc.vector.tensor_mul(
                        sq[:, 0:512],
                        xt[c][:, 512 * ch:512 * (ch + 1)],
                        xt[c][:, 512 * ch:512 * (ch + 1)])
                    nc.tensor.matmul(ssq[ch][0:1, 0:512], ones_col[:],
                                     sq[:, 0:512],
                                     start=(c == 0), stop=(c == CT - 1))
            rs_row = sb.tile([1, T], F32R, tag="rs_row", bufs=1)
            for ch in range(TCH):
                t0 = sb.tile([1, 512], F32, tag="t0", bufs=1)
                nc.vector.tensor_scalar(t0[:], ssq[ch][0:1, 0:512], 1.0 / C,
                                        EPS, op0=ALU.mult, op1=ALU.add)
                rec = sb.tile([1, 512], F32, tag="rec", bufs=1)
                nc.vector.reciprocal(rec[:], t0[:])
                # qsc = (1/(SH2*s_qkv))^2 folds x8/weight descale into rs
                nc.scalar.activation(rs_row[:, 512 * ch:512 * (ch + 1)], rec[:],
                                     A.Sqrt, scale=qsc_t[:, 0:1])
            rsb = sb.tile([128, T], F32, tag="rsb", bufs=1)
            for ch in range(TCH):
                p = ps.tile([128, 512], F32, tag="mm", bufs=2)
                nc.tensor.matmul(p[:], ones_row[:],
                                 rs_row[:, 512 * ch:512 * (ch + 1)],
                                 start=True, stop=True)
                nc.scalar.copy(rsb[:, 512 * ch:512 * (ch + 1)], p[:])
            rs_col = []
            for s in range(ST):
                p = ps.tile([128, 2], F32, tag="mm", bufs=2)
                nc.tensor.matmul(p[:], rs_row[:, 128 * s:128 * (s + 1)],
                                 ones_row[:, 0:2], start=True, stop=True)
                t = sb.tile([128, 1], F32, tag="rs_col", bufs=ST,
                            name=f"rs_col{s}")
                nc.scalar.copy(t[:], p[:, 0:1])
                rs_col.append(t)

            rsb_r = rsb[:, 512:1024]

            # V in fp8: vp8[sp] packs key-stile pair (2sp, 2sp+1) along DR
            # planes; per head 65 cols = 64 dims + ones (denominator).
            wv_t = {}
            for chd in range(TCH):
                wst = sb.tile([128, CP, 2, 512], F8, tag="wv8", bufs=TCH,
                              name=f"wv8_{chd}")
                nc.sync.dma_start(wst[:], wv8[chd])
                wv_t[chd] = wst
            E = DH + 1
            vp8 = [sb.tile([128, 2, H * E], F8, tag="vp8", bufs=ST // 2,
                           name=f"vp8_{sp}") for sp in range(ST // 2)]
            for sp in range(ST // 2):
                nc.vector.memset(
                    vp8[sp][:].rearrange("p i (h e) -> p (i h) e", e=E)
                    [:, :, DH:E], 1.0)
            for s in range(ST):
                v3 = vp8[s // 2][:, s % 2, :].rearrange(
                    "p (h e) -> p h e", e=E)
                for chd in range(TCH):
                    vp = ps.tile([128, 512], F32, tag="mm", bufs=2)
                    for j in range(CP):
                        nc.tensor.matmul(vp[:],
                                         x8[j][:, :, 128 * s:128 * (s + 1)],
                                         wv_t[chd][:, j],
                                         start=(j == 0), stop=(j == CP - 1),
                                         perf_mode=DR)
                    nc.vector.tensor_scalar(
                        v3[:, 8 * chd:8 * (chd + 1), 0:DH],
                        vp[:].rearrange("p (h d) -> p h d", d=DH),
                        rs_col[s][:], None, op0=ALU.mult)

            ot = [sb.tile([128, R], BF16, tag="otb", bufs=MT, name=f"ot{i}")
                  for i in range(MT)]
            pend = []

            def emit_av(h, m_, hh_, pg, pl, pd):
                av = ps.tile([DH + 1, R], F32, tag="av", bufs=2,
                             name=f"av{h}")
                vh = lambda sp: vp8[sp][:, :, E * h:E * (h + 1)]
                # lo cols 0:256: stile pairs (0,1) gated + (4,5) diag
                nc.tensor.matmul(av[:, 0:256], vh(0), pg[:, 0:2, :],
                                 start=True, stop=False, perf_mode=DR,
                                 skip_group_check=True)
                nc.tensor.matmul(av[:, 0:256], vh(2), pd[:, 0:2, :],
                                 start=False, stop=True, perf_mode=DR,
                                 skip_group_check=True)
                # hi cols 256:512: (0,1) plain + (2,3) gated + (4,5) plain
                # + (6,7) diag
                nc.tensor.matmul(av[:, 256:512], vh(0), pl[:, 0:2, :],
                                 start=False, stop=False, perf_mode=DR,
                                 skip_group_check=True)
                nc.tensor.matmul(av[:, 256:512], vh(1), pg[:, 2:4, :],
                                 start=False, stop=False, perf_mode=DR,
                                 skip_group_check=True)
                nc.tensor.matmul(av[:, 256:512], vh(2), pl[:, 2:4, :],
                                 start=False, stop=False, perf_mode=DR,
                                 skip_group_check=True)
                nc.tensor.matmul(av[:, 256:512], vh(3), pd[:, 2:4, :],
                                 start=False, stop=True, perf_mode=DR,
                                 skip_group_check=True)
                zrec = sb.tile([1, R], F32R, tag="zrec", bufs=2,
                               name=f"zrec{h}")
                nc.vector.reciprocal(zrec[:], av[DH:DH + 1, :])
                zbp = ps.tile([DH, R], F32, tag="mm", bufs=2, name=f"zbp{h}")
                nc.tensor.matmul(zbp[:], ones_row[:, 0:DH], zrec[:],
                                 start=True, stop=True)
                zb = sb.tile([DH, R], BF16, tag="zb", bufs=2, name=f"zb{h}")
                nc.vector.tensor_copy(zb[:], zbp[:])
                nc.vector.scalar_tensor_tensor(
                    ot[m_][64 * hh_:64 * (hh_ + 1), :], av[0:DH, :], 1.0,
                    zb[:], op0=ALU.mult, op1=ALU.mult)

            for m in range(MT):
                wstq = sb.tile([128, CP, 2, 128], F8, tag="wqk8", bufs=4,
                               name=f"wq{m}")
                nc.sync.dma_start(wstq[:], wq8[m])
                qp = ps.tile([128, R], F32, tag="mm", bufs=2)
                for j in range(CP):
                    nc.tensor.matmul(qp[:], wstq[:, j], x8r[j],
                                     start=(j == 0), stop=(j == CP - 1),
                                     perf_mode=DR)
                qt = tD(f"qt{m}")
                nc.vector.tensor_mul(qt[:], qp[:], rsb_r)

                wstk = sb.tile([128, CP, 2, 128], F8, tag="wqk8", bufs=4,
                               name=f"wk{m}")
                nc.sync.dma_start(wstk[:], wk8[m])
                kt = sb.tile([128, T], F32R, tag="kt", bufs=2, name=f"kt{m}")
                kpb = scb(f"kp{m}")
                for j in range(CP):
                    for ch in range(TCH):
                        nc.tensor.matmul(
                            kpb[:, 512 * ch:512 * (ch + 1)], wstk[:, j],
                            x8[j][:, :, 512 * ch:512 * (ch + 1)],
                            start=(j == 0), stop=(j == CP - 1),
                            perf_mode=DR, skip_group_check=True)
                nc.vector.tensor_mul(kt[:], kpb[:], rsb[:])

                for hh in range(2):
                    h = 2 * m + hh
                    q64 = qt[64 * hh:64 * (hh + 1), :]
                    k64 = lambda s: kt[64 * hh:64 * (hh + 1),
                                       128 * s:128 * (s + 1)]
                    qlo, qhi = q64[:, 0:256], q64[:, 256:512]
                    # grouped score slabs: 4 x [128 keys, 256 q] each;
                    # matmul->exp per slab so the 2-buffer scb pool pipelines
                    wg = scb(f"wg{h}")   # gated: s0lo s1lo s2hi s3hi
                    for i, (s, qq) in enumerate(
                            ((0, qlo), (1, qlo), (2, qhi), (3, qhi))):
                        nc.tensor.matmul(wg[:, 256 * i:256 * (i + 1)],
                                         k64(s), qq, start=True, stop=True)
                    pg = tP(f"pg{h}")
                    nc.scalar.activation(pg[:].rearrange("p i f -> p (i f)"),
                                         wg[:], A.Exp, scale=SCALE,
                                         bias=mbias_t[:, 0:1])
                    wl = scb(f"wl{h}")   # plain: s0hi s1hi s4hi s5hi
                    for i, (s, qq) in enumerate(
                            ((0, qhi), (1, qhi), (4, qhi), (5, qhi))):
                        nc.tensor.matmul(wl[:, 256 * i:256 * (i + 1)],
                                         k64(s), qq, start=True, stop=True)
                    pl = tP(f"pl{h}")
                    nc.scalar.activation(pl[:].rearrange("p i f -> p (i f)"),
                                         wl[:], A.Exp, scale=SCALE)
                    wd = scb(f"wd{h}")   # diag:  s4lo s5lo s6hi s7hi
                    for i, (s, qq, tt) in enumerate(
                            ((4, qlo, 0), (5, qlo, 1), (6, qhi, 0),
                             (7, qhi, 1))):
                        sl = wd[:, 256 * i:256 * (i + 1)]
                        nc.tensor.matmul(sl, k64(s), qq, start=True,
                                         stop=False, skip_group_check=True)
                        # += -1e30 * [key > q] (identity @ step pattern)
                        nc.tensor.matmul(sl, id_t[:],
                                         step_t[:, 256 * tt:256 * (tt + 1)],
                                         start=False, stop=True,
                                         skip_group_check=True)
                    pd = tP(f"pd{h}")
                    nc.scalar.activation(pd[:].rearrange("p i f -> p (i f)"),
                                         wd[:], A.Exp, scale=SCALE)
                    pend.append((h, m, hh, pg, pl, pd))
                    if len(pend) > 2:
                        emit_av(*pend.pop(0))

            while pend:
                emit_av(*pend.pop(0))

            xa = []
            for m in range(MT):
                wst = sb.tile([128, CT, 128], BF16, tag="wbig", bufs=2,
                              name=f"wp{m}")
                nc.sync.dma_start(wst[:], wpb[m])
                pp = ps.tile([128, R], F32, tag="mm", bufs=2)
                for c in range(CT):
                    nc.tensor.matmul(pp[:], wst[:, c, :], ot[c][:],
                                     start=(c == 0), stop=(c == CT - 1))
                t = tE(f"xa{m}")
                nc.vector.scalar_tensor_tensor(
                    t[:], pp[:], bp_t[:, m:m + 1], xtr[m],
                    op0=ALU.add, op1=ALU.add)
                xa.append(t)

            ssq2 = ps.tile([1, R], F32, tag="mm", bufs=2)
            for c in range(CT):
                sq2 = tD(f"sq2_{c}")
                nc.vector.tensor_mul(sq2[:], xa[c][:], xa[c][:])
                nc.tensor.matmul(ssq2[:], ones_col[:], sq2[:],
                                 start=(c == 0), stop=(c == CT - 1))
            t02 = sb.tile([1, R], F32, tag="t0", bufs=1)
            nc.vector.tensor_scalar(t02[:], ssq2[:], 1.0 / C, EPS,
                                    op0=ALU.mult, op1=ALU.add)
            rec2 = sb.tile([1, R], F32, tag="rec", bufs=1)
            nc.vector.reciprocal(rec2[:], t02[:])
            rs2_row = sb.tile([1, R], F32R, tag="rs2_row", bufs=1)
            nc.scalar.activation(rs2_row[:], rec2[:], A.Sqrt)
            rsb2 = tE("rsb2")
            p2 = ps.tile([128, R], F32, tag="mm", bufs=2)
            nc.tensor.matmul(p2[:], ones_row[:], rs2_row[:], start=True, stop=True)
            nc.scalar.copy(rsb2[:], p2[:])
            h2 = [sb.tile([128, R], BF16, tag="h2", bufs=CT, name=f"h2_{c}")
                  for c in range(CT)]
            for c in range(CT):
                nc.vector.tensor_mul(h2[c][:], xa[c][:], rsb2[:])

            for m in range(MT):
                nc.vector.tensor_scalar(xa[m][:], xa[m][:], b2_t[:, m:m + 1],
                                        None, op0=ALU.add)
            g = [sb.tile([128, R], BF16, tag="g", bufs=FT, name=f"g_{d}")
                 for d in range(FT)]
            for dp in range(FT // 2):
                w1t = sb.tile([128, 2, CT, 128], BF16, tag="w1b", bufs=3,
                              name=f"w1_{dp}")
                nc.sync.dma_start(
                    w1t[:], w1b.rearrange("(f two) p ct n -> f p two ct n",
                                          two=2)[dp])
                fp2 = scb(f"fp{dp}")
                for half in range(2):
                    for c in range(CT):
                        nc.tensor.matmul(
                            fp2[:, 512 * half:512 * (half + 1)],
                            w1t[:, half, c, :], h2[c][:],
                            start=(c == 0), stop=(c == CT - 1),
                            skip_group_check=True)
                for half in range(2):
                    d = 2 * dp + half
                    nc.scalar.activation(g[d][:],
                                         fp2[:, 512 * half:512 * (half + 1)],
                                         A.Relu, bias=b1_t[:, d:d + 1])
            for m in range(MT):
                w2t = sb.tile([128, FT, 128], BF16, tag="w2b", bufs=2,
                              name=f"w2_{m}")
                nc.sync.dma_start(w2t[:], w2b[m])
                pp = ps.tile([128, R], F32, tag="mm", bufs=2)
                for di in range(FT):
                    nc.tensor.matmul(pp[:], w2t[:, di, :], g[di][:],
                                     start=(di == 0), stop=(di == FT - 1))
                o = tE(f"out{m}")
                nc.vector.scalar_tensor_tensor(
                    o[:], pp[:], 0.0, xa[m][:], op0=ALU.add, op1=ALU.add)
                nc.sync.dma_start(outT[128 * m:128 * (m + 1), :], o[:])

        if repeat == 1:
            body()
        else:
            with tc.For_i(0, repeat, 1):
                body()

    nc.compile()
    return nc


def _step_masks() -> np.ndarray:
    """[128, 512] f32: cols 256t+q hold -1e30 where 128t + a > q (the
    additive causal bias pattern for diag slabs with key-stile parity t)."""
    qq = np.arange(256)
    aa = np.arange(128)
    out = np.zeros((128, 512), np.float32)
    for t in range(2):
        out[:, 256 * t:256 * (t + 1)] = np.where(
            (128 * t + aa[:, None]) > qq[None, :], -1e30, 0.0)
    return out


def _prepare_in_maps(inputs: dict) -> list[dict]:
    import ml_dtypes
    x = np.asarray(inputs["x"], np.float32)
    g1 = np.asarray(inputs["g1"], np.float32)
    g2 = np.asarray(inputs["g2"], np.float32)
    wq = (g1[:, None, None] * np.asarray(inputs["wq"], np.float32)).reshape(C, C)
    wk = (g1[:, None, None] * np.asarray(inputs["wk"], np.float32)).reshape(C, C)
    wv = (g1[:, None, None] * np.asarray(inputs["wv"], np.float32)).reshape(C, C)
    wp = np.ascontiguousarray(np.asarray(inputs["w_proj"], np.float32))
    w1 = np.ascontiguousarray(g2[:, None] * np.asarray(inputs["w1"], np.float32))
    w2 = np.ascontiguousarray(np.asarray(inputs["w2"], np.float32))
    bf = lambda a: np.ascontiguousarray(
        a.astype(ml_dtypes.bfloat16).view(np.uint16))
    # w1: [C, DFF] -> [FT*2half, 128p, CT, 128col] bf16  (dram [FT,...] pairs)
    w1b = bf(w1.reshape(CT, 128, FT, 128).transpose(2, 1, 0, 3))
    # w2: [DFF, C] -> [MT, 128p, FT, 128col] bf16
    w2b = bf(w2.reshape(FT, 128, MT, 128).transpose(2, 1, 0, 3))
    # qkv weights: single fp8 at one shared p2 scale
    mq = max(np.abs(wq).max(), np.abs(wk).max(), np.abs(wv).max())
    sq = 2.0 ** np.floor(np.log2(224.0 / mq))
    q8c = lambda a: np.ascontiguousarray(
        (a * sq).astype(ml_dtypes.float8_e4m3).view(np.uint8))
    wq8 = q8c(wq.reshape(CP, 2, 128, MT, 128).transpose(3, 2, 0, 1, 4))
    wk8 = q8c(wk.reshape(CP, 2, 128, MT, 128).transpose(3, 2, 0, 1, 4))
    wv8 = q8c(wv.reshape(CP, 2, 128, TCH, 512).transpose(3, 2, 0, 1, 4))
    shared = {
        "wq8": wq8, "wk8": wk8, "wv8": wv8,
        "qsc": np.full(1, (1.0 / (SH2 * sq)) ** 2, np.float32),
        "wpb": bf(wp.reshape(CT, 128, MT, 128).transpose(2, 1, 0, 3)),
        "w1b": w1b, "w2b": w2b,
        "bp": np.asarray(inputs["b_proj"], np.float32),
        "b1": np.asarray(inputs["b1"], np.float32),
        "b2": np.asarray(inputs["b2"], np.float32),
        "ident": np.eye(128, dtype=np.float32),
        "stepm": _step_masks(),
        "ones": np.ones(128, np.float32),
    }
    in_maps = []
    for core in range(N_CORES):
        b, half = core // 2, core % 2
        xT_full = x[b].T
        blk = lambda gidx: xT_full[:, gidx:gidx + 256]
        o_lo, o_hi = blk(256 * (1 - half)), blk(512 + 256 * (1 - half))
        s_lo, s_hi = blk(256 * half), blk(512 + 256 * half)
        xT_perm = np.ascontiguousarray(
            np.concatenate([o_lo, o_hi, s_lo, s_hi], axis=1))
        mb = np.zeros((128, ST), np.float32)
        if half == 0:
            mb[:, 0] = -1e30   # gated slab group: other rows are future
        in_maps.append({
            "xT": bf(xT_perm),
            "mbias": mb,
            **shared,
        })
    return in_maps


_RUNNER_CACHE: dict = {}


def make_runner(repeat: int = 1):
    """Compile the module; return (run_fn, put_inputs, nc)."""
    if repeat in _RUNNER_CACHE:
        return _RUNNER_CACHE[repeat]
    from jax.sharding import Mesh, PartitionSpec, NamedSharding
    from jax.experimental.shard_map import shard_map

    nc = build_module(repeat)
    bass2jax.install_neuronx_cc_hook()
    partition_name = nc.partition_id_tensor.name if nc.partition_id_tensor else None
    in_names, out_names, out_avals, zero_shapes = [], [], [], []
    for alloc in nc.m.functions[0].allocations:
        if not isinstance(alloc, mybir.MemoryLocationSet):
            continue
        name = alloc.memorylocations[0].name
        if alloc.kind == "ExternalInput":
            if name != partition_name:
                in_names.append(name)
        elif alloc.kind == "ExternalOutput":
            out_names.append(name)
            shape = tuple(alloc.tensor_shape)
            dtype = mybir.dt.np(alloc.dtype)
            out_avals.append(jax.core.ShapedArray(shape, dtype))
            zero_shapes.append((shape, dtype))
    n_params = len(in_names)
    all_in_names = in_names + out_names + ([partition_name] if partition_name else [])

    def _body(*args):
        operands = list(args)
        if partition_name is not None:
            operands.append(bass2jax.partition_id_tensor())
        return tuple(bass2jax._bass_exec_p.bind(
            *operands, out_avals=tuple(out_avals), in_names=tuple(all_in_names),
            out_names=tuple(out_names), lowering_input_output_aliases=(),
            sim_require_finite=True, sim_require_nnan=True, nc=nc))

    devices = jax.devices()[:N_CORES]
    mesh = Mesh(np.asarray(devices), ("core",))
    nin = n_params + len(out_names)
    sharded = jax.jit(
        shard_map(_body, mesh=mesh, in_specs=(PartitionSpec("core"),) * nin,
                  out_specs=(PartitionSpec("core"),) * len(out_names),
                  check_rep=False),
        keep_unused=True)
    sharding = NamedSharding(mesh, PartitionSpec("core"))

    def put_inputs(in_maps):
        concat_in = [np.concatenate([np.asarray(in_maps[c][n])
                                     for c in range(N_CORES)], axis=0)
                     for n in in_names]
        concat_zeros = [np.zeros((N_CORES * s[0], *s[1:]), d)
                        for (s, d) in zero_shapes]
        return [jax.device_put(a, sharding) for a in concat_in + concat_zeros]

    def run_fn(in_maps=None, device_args=None):
        if device_args is None:
            device_args = put_inputs(in_maps)
        out_arrs = sharded(*device_args)
        jax.block_until_ready(out_arrs)
        return [
            {name: np.asarray(out_arrs[i]).reshape(N_CORES, *out_avals[i].shape)[c]
             for i, name in enumerate(out_names)}
            for c in range(N_CORES)
        ]

    _RUNNER_CACHE[repeat] = (run_fn, put_inputs, nc)
    return _RUNNER_CACHE[repeat]


def kernel(**inputs) -> np.ndarray:
    run_fn, _put, _nc = make_runner(repeat=1)
    in_maps = _prepare_in_maps(inputs)
    results = run_fn(in_maps)
    out = np.empty((B, T, C), np.float32)
    for core in range(N_CORES):
        b, half = core // 2, core % 2
        res = results[core]["outT"]
        out[b, 256 * half:256 * half + 256, :] = res[:, 0:256].T
        out[b, 512 + 256 * half:512 + 256 * half + 256, :] = res[:, 256:512].T
    return out
